# revision 9
# baseline (speedup 1.0000x reference)
"""GCN message-passing (nn_Discriminator) on 8 Trainium2 NeuronCores.

Algorithm: per layer, h_new = (A@h)@Wm_h + h@Ws + segsum(edge_attr)@Wm_e + b
(the per-edge matmul commutes with segment_sum). A@h is computed per core
(nodes sharded by dst) as one-hot scatter matmuls on TensorE over edge tiles
sorted by dst window, with h rows fetched by dma_gather (4 SWDGE queues in
parallel). One-hot R matrices are host-built and streamed from HBM.

Between layers, node features are AllGathered (bf16) in TWO region
collectives: C1 covers each core's first 25 windows and fires mid-layer, so
the next layer's region-1 gathers overlap the current layer's tail; C2 covers
the rest. Each layer runs two passes: pass 1 accumulates region-1 edge
contributions into an SBUF buffer, pass 2 adds region-2 contributions and
runs the node-level matmuls/epilogue.

Sharding: nodes (and edges by dst) across 8 cores; weights replicated.
"""
import numpy as np
import ml_dtypes

bf16 = ml_dtypes.bfloat16

N_NODES = 50000
N_EDGES = 800000
N_CORES = 8
NPC = N_NODES // N_CORES          # 6250 real nodes per core
P = 128
NWIN = (NPC + P - 1) // P         # 49 windows
NL = NWIN * P                     # 6272 padded nodes per core
SPLW = 25                         # windows in region 1
SPL = SPLW * P                    # 3200 local rows in region 1
R1 = SPL * N_CORES                # 25600 region-1 positions
R2 = (NL - SPL) * N_CORES         # 24576 region-2 positions
CH = 32                           # gather chunk, tiles per dma_gather call
RCH = 16                          # R-stream chunk, tiles per DMA
DIMS = [32, 64, 128, 128, 128, 128]
DIN_S = DIMS[:5]
DOUT = DIMS[1:]

_cache = {}


def _positions(g):
    """Gather position of global node id g under the region layout."""
    r = g // NPC
    l = g % NPC
    return np.where(l < SPL, SPL * r + l, R1 + (NL - SPL) * r + (l - SPL))


def _build_schedule(src, dst):
    src = np.asarray(src).astype(np.int64)
    dst = np.asarray(dst).astype(np.int64)
    rank = dst // NPC
    dloc = dst % NPC
    w = dloc // P
    spos = _positions(src)
    half = (spos >= R1).astype(np.int64)

    order = np.lexsort((spos, half, w, rank))
    rs, ws, hs = rank[order], w[order], half[order]
    counts = np.zeros((N_CORES, NWIN, 2), np.int64)
    np.add.at(counts, (rs, ws, hs), 1)
    ntiles = (-(-counts // P)).max(axis=0)       # [NWIN, 2]
    ntiles = np.maximum(ntiles, 1)               # every (window, region) present

    # pass-major schedule: all region-1 groups (window order), then region-2
    sched = []
    for h in (0, 1):
        for wi in range(NWIN):
            sched.append((wi, h, int(ntiles[wi, h])))

    key = (rs * NWIN + ws) * 2 + hs
    boundaries = np.searchsorted(key, np.arange(N_CORES * NWIN * 2 + 1))
    per_core = []
    for c in range(N_CORES):
        posA, posB, dstw_all, eidx_all = [], [], [], []
        for (wi, h, nt) in sched:
            k = (c * NWIN + wi) * 2 + h
            lo, hi = boundaries[k], boundaries[k + 1]
            e = order[lo:hi]
            npad = nt * P - (hi - lo)
            ps = np.concatenate([spos[e] - (R1 if h else 0),
                                 np.zeros(npad, np.int64)])
            dl = np.concatenate([dloc[e] - wi * P, -np.ones(npad, np.int64)])
            ei = np.concatenate([e, -np.ones(npad, np.int64)])
            (posA if h == 0 else posB).append(ps)
            dstw_all.append(dl)
            eidx_all.append(ei)
        per_core.append(dict(
            posA=np.concatenate(posA).astype(np.int32),
            posB=np.concatenate(posB).astype(np.int32),
            dstw=np.concatenate(dstw_all).astype(np.int32),
            eidx=np.concatenate(eidx_all).astype(np.int64),
        ))
    return sched, per_core


def _wrap_idx(pos):
    n = len(pos)
    img = np.zeros((128, max(n // 16, 1)), np.int16)
    if n:
        img[np.arange(n) % 16, np.arange(n) // 16] = pos.astype(np.int16)
        for g in range(1, 8):
            img[g * 16:(g + 1) * 16] = img[:16]
    return img


def _build_program(sched, TA, TB):
    from concourse import mybir, bacc
    import concourse.tile as tile

    T = sum(nt for _, _, nt in sched)
    nc = bacc.Bacc("TRN2", target_bir_lowering=False, debug=False,
                   num_devices=N_CORES, num_swdge_queues=4)
    dt = mybir.dt

    x1_d = nc.dram_tensor("x1", [R1, 128], dt.bfloat16, kind="ExternalInput")
    x2_d = nc.dram_tensor("x2", [R2, 128], dt.bfloat16, kind="ExternalInput")
    xT_in = nc.dram_tensor("xT", [32, NL], dt.bfloat16, kind="ExternalInput")
    idxA_d = nc.dram_tensor("idxA", [128, TA * 8], dt.int16, kind="ExternalInput")
    idxB_d = nc.dram_tensor("idxB", [128, TB * 8], dt.int16, kind="ExternalInput")
    rimg_d = nc.dram_tensor("rimg", [128, T * 128], dt.bfloat16, kind="ExternalInput")
    ea_d = nc.dram_tensor("eaimg", [128, T * 8], dt.bfloat16, kind="ExternalInput")
    ident_d = nc.dram_tensor("ident", [128, 128], dt.bfloat16, kind="ExternalInput")
    wmh_d, ws_d, wme_d, b_d = [], [], [], []
    for l in range(5):
        wmh_d.append(nc.dram_tensor(f"Wmh{l}", [DIN_S[l], DOUT[l]], dt.bfloat16, kind="ExternalInput"))
        ws_d.append(nc.dram_tensor(f"Wsb{l}", [DIN_S[l], DOUT[l]], dt.bfloat16, kind="ExternalInput"))
        wme_d.append(nc.dram_tensor(f"Wme{l}", [8, DOUT[l]], dt.bfloat16, kind="ExternalInput"))
        b_d.append(nc.dram_tensor(f"bb{l}", [DOUT[l], 1], dt.float32, kind="ExternalInput"))
    wc_d = nc.dram_tensor("Wcb", [128, 1], dt.bfloat16, kind="ExternalInput")
    bc_d = nc.dram_tensor("bcb", [128, 1], dt.float32, kind="ExternalInput")
    out_d = nc.dram_tensor("out", [NL, 1], dt.float32, kind="ExternalOutput")

    # per-pass tile counts per window
    ntile_w = {(wi, h): nt for (wi, h, nt) in sched}

    with tile.TileContext(nc) as tc:
        with tc.tile_pool(name="consts", bufs=1) as consts, \
             tc.tile_pool(name="gxa", bufs=5) as gxa_pool, \
             tc.tile_pool(name="gxb", bufs=5) as gxb_pool, \
             tc.tile_pool(name="rst", bufs=3) as rst_pool, \
             tc.tile_pool(name="idc", bufs=4) as idc_pool, \
             tc.tile_pool(name="eac", bufs=3) as eac_pool, \
             tc.tile_pool(name="wk", bufs=3) as wk, \
             tc.tile_pool(name="hT", bufs=2) as hT_pool, \
             tc.tile_pool(name="a1p", bufs=2, space="PSUM") as a1_ps, \
             tc.tile_pool(name="a2p", bufs=2, space="PSUM") as a2_ps, \
             tc.tile_pool(name="aep", bufs=1, space="PSUM") as ae_ps, \
             tc.tile_pool(name="hnp", bufs=2, space="PSUM") as hn_ps, \
             tc.tile_pool(name="trp", bufs=1, space="PSUM") as tr_ps, \
             tc.tile_pool(name="dram", bufs=1, space="DRAM") as dram:

            # ---- resident constants ----
            ident = consts.tile([128, 128], dt.bfloat16)
            nc.sync.dma_start(out=ident[:], in_=ident_d[:])
            wmh, wsb, wme, bb = [], [], [], []
            for l in range(5):
                t1 = consts.tile([DIN_S[l], DOUT[l]], dt.bfloat16, tag=f"wmh{l}")
                t2 = consts.tile([DIN_S[l], DOUT[l]], dt.bfloat16, tag=f"wsb{l}")
                t3 = consts.tile([8, DOUT[l]], dt.bfloat16, tag=f"wme{l}")
                t4 = consts.tile([DOUT[l], 1], dt.float32, tag=f"bb{l}")
                nc.sync.dma_start(out=t1[:], in_=wmh_d[l][:])
                nc.sync.dma_start(out=t2[:], in_=ws_d[l][:])
                nc.sync.dma_start(out=t3[:], in_=wme_d[l][:])
                nc.sync.dma_start(out=t4[:], in_=b_d[l][:])
                wmh.append(t1); wsb.append(t2); wme.append(t3); bb.append(t4)
            wc = consts.tile([128, 1], dt.bfloat16)
            bcb = consts.tile([128, 1], dt.float32)
            nc.sync.dma_start(out=wc[:], in_=wc_d[:])
            nc.sync.dma_start(out=bcb[:], in_=bc_d[:])
            aeT = consts.tile([8, NL], dt.bfloat16)
            a1sb = consts.tile([128, NL], dt.bfloat16)

            cc1_in = [dram.tile([SPL, 128], dt.bfloat16, tag=f"c1i{l}", name=f"c1i{l}")
                      for l in range(4)]
            cc1_out = [dram.tile([R1, 128], dt.bfloat16, tag=f"c1o{l}", name=f"c1o{l}")
                       for l in range(4)]
            cc2_in = [dram.tile([NL - SPL, 128], dt.bfloat16, tag=f"c2i{l}", name=f"c2i{l}")
                      for l in range(4)]
            cc2_out = [dram.tile([R2, 128], dt.bfloat16, tag=f"c2o{l}", name=f"c2o{l}")
                       for l in range(4)]

            hT0 = hT_pool.tile([128, NL], dt.bfloat16, tag="hT", name="hT0")
            nc.sync.dma_start(out=hT0[:32, :], in_=xT_in[:])
            hT_cur = hT0
            for l in range(5):
                din, dout = DIN_S[l], DOUT[l]
                srcA = x1_d if l == 0 else cc1_out[l - 1]
                srcB = x2_d if l == 0 else cc2_out[l - 1]
                stream_src = {"A": srcA[:, :], "B": srcB[:, :]}
                stream_idx = {"A": idxA_d, "B": idxB_d}
                stream_T = {"A": TA, "B": TB}
                gx_pool = {"A": gxa_pool, "B": gxb_pool}
                gx_tiles = {"A": {}, "B": {}}
                r_tiles = {}
                qrr = [0]

                def slot(S, j):
                    ch = j // CH
                    tl = gx_tiles[S]
                    if ch not in tl:
                        nt = min(CH, stream_T[S] - ch * CH)
                        g = gx_pool[S].tile([128, nt, 128], dt.bfloat16, tag="gx" + S)
                        ic = idc_pool.tile([128, nt * 8], dt.int16, tag="idc")
                        nc.sync.dma_start(
                            out=ic[:],
                            in_=stream_idx[S][:, ch * CH * 8:(ch * CH + nt) * 8])
                        nc.gpsimd.dma_gather(
                            out_ap=g[:],
                            in_ap=stream_src[S],
                            idxs_ap=ic[:],
                            num_idxs=nt * 128,
                            num_idxs_reg=nt * 128,
                            elem_size=128,
                            single_packet=False,
                            queue_num=qrr[0],
                        )
                        qrr[0] = (qrr[0] + 1) % 4
                        tl[ch] = g
                    return tl[ch][:, j % CH, :]

                def rslot(t):
                    ch = t // RCH
                    if ch not in r_tiles:
                        nt = min(RCH, T - ch * RCH)
                        r = rst_pool.tile([128, nt * 128], dt.bfloat16, tag="rch")
                        nc.sync.dma_start(
                            out=r[:],
                            in_=rimg_d[:, ch * RCH * 128:(ch * RCH + nt) * 128])
                        if l == 0:
                            e = eac_pool.tile([128, nt * 8], dt.bfloat16, tag="eac")
                            nc.sync.dma_start(
                                out=e[:],
                                in_=ea_d[:, ch * RCH * 8:(ch * RCH + nt) * 8])
                            r_tiles[ch] = (r, e)
                        else:
                            r_tiles[ch] = (r, None)
                    k = t % RCH
                    r, e = r_tiles[ch]
                    return r[:, k * 128:(k + 1) * 128], (e[:, k * 8:(k + 1) * 8] if e is not None else None)

                hT_next = hT_pool.tile([128, NL], dt.bfloat16, tag="hT")
                if l == 0:
                    nc.vector.memset(hT_next[64:128, :], 0)

                a_ctr = b_ctr = 0
                t_global = 0
                # ---- pass 1: region-1 edge tiles, all windows ----
                for wi in range(NWIN):
                    nt = ntile_w[(wi, 0)]
                    a1 = a1_ps.tile([din, 128], dt.float32, tag="a1", space="PSUM")
                    if l == 0:
                        ae = ae_ps.tile([8, 128], dt.float32, tag="ae", space="PSUM")
                    for j in range(nt):
                        xs = slot("A", a_ctr); a_ctr += 1
                        R, easl = rslot(t_global)
                        nc.tensor.matmul(out=a1[:], lhsT=xs[:, :din], rhs=R,
                                         start=(j == 0), stop=(j == nt - 1))
                        if l == 0:
                            nc.tensor.matmul(
                                out=ae[:], lhsT=easl,
                                rhs=R, start=(j == 0), stop=(j == nt - 1))
                        t_global += 1
                    wsl = slice(wi * P, (wi + 1) * P)
                    nc.vector.tensor_copy(out=a1sb[:din, wsl], in_=a1[:])
                    if l == 0:
                        nc.vector.tensor_copy(out=aeT[:, wsl], in_=ae[:])

                # ---- pass 2: region-2 edge tiles + epilogues ----
                for wi in range(NWIN):
                    nt = ntile_w[(wi, 1)]
                    a2 = a2_ps.tile([din, 128], dt.float32, tag="a2", space="PSUM")
                    if l == 0:
                        ae = ae_ps.tile([8, 128], dt.float32, tag="ae", space="PSUM")
                    for j in range(nt):
                        xs = slot("B", b_ctr); b_ctr += 1
                        R, easl = rslot(t_global)
                        nc.tensor.matmul(out=a2[:], lhsT=xs[:, :din], rhs=R,
                                         start=(j == 0), stop=(j == nt - 1))
                        if l == 0:
                            nc.tensor.matmul(
                                out=ae[:], lhsT=easl,
                                rhs=R, start=(j == 0), stop=(j == nt - 1))
                        t_global += 1
                    wsl = slice(wi * P, (wi + 1) * P)
                    a2sb = wk.tile([din, 128], dt.bfloat16, tag="a2sb")
                    nc.vector.tensor_copy(out=a2sb[:], in_=a2[:])
                    if l == 0:
                        nc.vector.tensor_tensor(
                            out=aeT[:, wsl], in0=aeT[:, wsl], in1=ae[:],
                            op=mybir.AluOpType.add)
                    hn = hn_ps.tile([dout, 128], dt.float32, tag="hn", space="PSUM")
                    nc.tensor.matmul(out=hn[:], lhsT=wmh[l][:], rhs=a1sb[:din, wsl],
                                     start=True, stop=False)
                    nc.tensor.matmul(out=hn[:], lhsT=wmh[l][:], rhs=a2sb[:],
                                     start=False, stop=False)
                    nc.tensor.matmul(out=hn[:], lhsT=wsb[l][:],
                                     rhs=hT_cur[:din, wsl], start=False, stop=False)
                    nc.tensor.matmul(out=hn[:], lhsT=wme[l][:], rhs=aeT[:, wsl],
                                     start=False, stop=True)
                    nc.vector.tensor_scalar(
                        out=hT_next[:dout, wsl], in0=hn[:],
                        scalar1=bb[l][:], scalar2=0.0,
                        op0=mybir.AluOpType.add, op1=mybir.AluOpType.max)
                    if l < 4:
                        tr = tr_ps.tile([128, 128], dt.bfloat16, tag="tr", space="PSUM")
                        nc.tensor.transpose(out=tr[:], in_=hT_next[:, wsl], identity=ident[:])
                        rows = wk.tile([128, 128], dt.bfloat16, tag="rows")
                        nc.vector.tensor_copy(out=rows[:], in_=tr[:])
                        if wi < SPLW:
                            nc.sync.dma_start(out=cc1_in[l][wsl, :], in_=rows[:])
                        else:
                            rsl = slice(wi * P - SPL, (wi + 1) * P - SPL)
                            nc.sync.dma_start(out=cc2_in[l][rsl, :], in_=rows[:])
                        if wi == SPLW - 1:
                            nc.gpsimd.collective_compute(
                                "AllGather", mybir.AluOpType.bypass,
                                replica_groups=[list(range(N_CORES))],
                                ins=[cc1_in[l].opt()], outs=[cc1_out[l].opt()])
                        if wi == NWIN - 1:
                            nc.gpsimd.collective_compute(
                                "AllGather", mybir.AluOpType.bypass,
                                replica_groups=[list(range(N_CORES))],
                                ins=[cc2_in[l].opt()], outs=[cc2_out[l].opt()])
                    else:
                        hd = tr_ps.tile([128, 1], dt.float32, tag="tr", space="PSUM")
                        nc.tensor.matmul(out=hd[:], lhsT=hT_next[:, wsl], rhs=wc[:],
                                         start=True, stop=True)
                        hd_sb = wk.tile([128, 1], dt.float32, tag="hd_sb")
                        nc.vector.tensor_scalar(
                            out=hd_sb[:], in0=hd[:], scalar1=bcb[:], scalar2=None,
                            op0=mybir.AluOpType.add)
                        nc.sync.dma_start(out=out_d[wsl, :], in_=hd_sb[:])

                hT_cur = hT_next

    nc.finalize()
    return nc


def kernel(**inputs):
    from concourse.bass_utils import run_bass_kernel_spmd

    src = np.asarray(inputs["src"]).astype(np.int64)
    dst = np.asarray(inputs["dst"]).astype(np.int64)
    sched, per_core = _build_schedule(src, dst)
    T = sum(nt for _, _, nt in sched)
    TA = sum(nt for _, h, nt in sched if h == 0)
    TB = T - TA

    key = ("v3b", T, TA, tuple(nt for _, _, nt in sched))
    if key not in _cache:
        _cache.clear()
        _cache[key] = _build_program(sched, TA, TB)
    nc = _cache[key]

    x = np.asarray(inputs["x"], np.float32)
    ea = np.asarray(inputs["edge_attr"], np.float32)
    # positional layout of x (region split)
    x_pos = np.zeros((R1 + R2, 128), bf16)
    g = np.arange(N_NODES)
    x_pos[_positions(g), :32] = x.astype(bf16)
    ident = np.eye(128, dtype=np.float32).astype(bf16)

    shared = {"x1": x_pos[:R1], "x2": x_pos[R1:], "ident": ident}
    for l in range(5):
        Wm = np.asarray(inputs[f"Wm{l}"], np.float32)
        shared[f"Wmh{l}"] = Wm[:DIN_S[l]].astype(bf16)
        shared[f"Wme{l}"] = Wm[DIN_S[l]:].astype(bf16)
        shared[f"Wsb{l}"] = np.asarray(inputs[f"Ws{l}"], np.float32).astype(bf16)
        shared[f"bb{l}"] = np.asarray(inputs[f"b{l}"], np.float32).reshape(-1, 1)
    shared["Wcb"] = np.asarray(inputs["Wc"], np.float32).astype(bf16)
    shared["bcb"] = np.full((128, 1), np.asarray(inputs["bc"], np.float32).reshape(-1)[0], np.float32)

    in_maps = []
    for c in range(N_CORES):
        pc = per_core[c]
        dstw = pc["dstw"]
        rimg = np.zeros((128, T * 128), bf16)
        sl = np.arange(T * 128)
        m = dstw >= 0
        rimg[sl[m] % 128, (sl[m] // 128) * 128 + dstw[m]] = 1
        ei = pc["eidx"]
        eav = np.zeros((T * 128, 8), np.float32)
        me = ei >= 0
        eav[me] = ea[ei[me]]
        ea_img = np.ascontiguousarray(
            eav.reshape(T, 128, 8).transpose(1, 0, 2).reshape(128, T * 8)).astype(bf16)
        # local transposed x: local row l of core c is at position
        xT_img = np.zeros((32, NL), bf16)
        lg = np.arange(NPC)
        xT_img[:, lg] = x.astype(bf16)[c * NPC + lg].T
        in_maps.append({
            **shared,
            "idxA": _wrap_idx(pc["posA"]),
            "idxB": _wrap_idx(pc["posB"]),
            "rimg": rimg,
            "eaimg": ea_img,
            "xT": xT_img,
        })

    res = run_bass_kernel_spmd(nc, in_maps, core_ids=list(range(N_CORES)))
    out = np.concatenate([res.results[c]["out"][:NPC] for c in range(N_CORES)], axis=0)
    return out.astype(np.float32)
